# revision 2
# baseline (speedup 1.0000x reference)
"""3x3 median blur on Trainium2: hybrid stock-2x / custom-1x DVE kernel.

Self-contained: custom DVE uop programs (hand-written, registered into
concourse.dve_ops at build time) + the bass kernel. See class/function
docstrings for the design.
"""


from dataclasses import dataclass, field

import numpy as np

import concourse.dve_ops as dve_ops
from concourse.dve_spec import Spec, Src0, Src1
from concourse.dve_uop import (
    AluInp,
    AluOp,
    DelayInp,
    InpSel,
    OutPath,
    OutSel,
    Trigger,
    UopConfig,
    DveOpSpec,
)

PD = [AluInp.PREV_DELAY_0, AluInp.PREV_DELAY_1, AluInp.PREV_DELAY_2,
      AluInp.PREV_DELAY_3, AluInp.PREV_DELAY_4, AluInp.PREV_DELAY_5]
PREV = AluInp.PREV_ALU_OUT
NEXTA = AluInp.NEXT_ALU_OUT_A


@dataclass(frozen=True)
class RawDveOp:
    """DveOp-alike whose compile() returns hand-written uops directly."""

    name: str
    spec: Spec          # placeholder body with the right leaves (sim only)
    uops: tuple        # tuple[UopConfig, ...]
    rd1_en: bool
    subdim: bool = False
    perf_en: dict = field(default_factory=dict)

    def compile(self, ver):
        assert ver == "v3", f"hand uops are v3-only (got {ver})"
        return DveOpSpec(
            name=self.name,
            opcode=dve_ops.get_dve_sub_opcode(self.name),
            uops=list(self.uops),
            rd1_en=self.rd1_en,
        )


def _steady(rd1: bool) -> UopConfig:
    u = UopConfig()
    u.trigger = (Trigger.SRC_TENSOR_DONE, Trigger.NONE, Trigger.NONE)
    u.next_uop = (0, 0, 0)
    u.require_inp0 = 1
    u.require_inp1 = 1 if rd1 else 0
    return u


def _finish_bypass(u: UopConfig, first_unused: int, carry: tuple = ()):
    """Blocks first_unused..7 pass ALU result (and listed delay chains)."""
    for k in range(first_unused, 8):
        u.datapath_config[k].pass_through_alu()
        if carry:
            u.datapath_config[k].pass_through_delay(*carry)


def build_v1() -> UopConfig:
    # lanes: 1 = SRC_0 (b = X[r]), 2 = SRC_1 (c = X[r+1]); a = X[r-1]
    u = _steady(rd1=True)
    u.enable_input(InpSel.SRC_0, 1)
    u.enable_input(InpSel.SRC_1, 2)
    B = u.datapath_config
    # blk0: p = min(a, b)
    B[0].enable_alu(AluOp.MIN, NEXTA, PD[0])
    B[0].pass_through_delay(0, 1)            # c0 <- b, c1 <- c
    B[0].enable_delay_from_src(DelayInp.NEXT_ALU_OUT_A, 2)   # c2 <- a
    # blk1: provider BYPASS(c) with a-flop
    B[1].enable_alu(AluOp.BYPASS, PD[1])
    B[1].alu_out_a_enable = 1
    B[1].pass_through_delay(0, 1, 2)
    B[1].enable_delay_from_src(DelayInp.PREV_ALU_OUT, 3)     # c3 <- p
    # blk2: q = max(a, b)
    B[2].enable_alu(AluOp.MAX, PD[2], PD[0])
    B[2].pass_through_delay(1, 3)
    # blk3: lo = min(p, c)
    B[3].enable_alu(AluOp.MIN, PD[3], PD[1])
    B[3].pass_through_delay(1)
    B[3].enable_delay_from_src(DelayInp.PREV_ALU_OUT, 4)     # c4 <- q
    # blk4: hi = max(q, c)
    B[4].enable_alu(AluOp.MAX, PD[4], PD[1])
    B[4].enable_delay_from_src(DelayInp.PREV_ALU_OUT, 0)     # c0 <- lo
    _finish_bypass(u, 5, carry=(0,))
    u.enable_output(OutSel.DELAY_0, OutPath.WR0_LO)   # lo
    u.enable_output(OutSel.ALU_OUT, OutPath.WR0_HI)   # hi
    return u


def build_v2() -> UopConfig:
    # lanes as V1; out = mid = med3(a, b, c)
    u = _steady(rd1=True)
    u.enable_input(InpSel.SRC_0, 1)
    u.enable_input(InpSel.SRC_1, 2)
    B = u.datapath_config
    B[0].enable_alu(AluOp.MIN, NEXTA, PD[0])                 # p = min(a,b)
    B[0].pass_through_delay(0, 1)
    B[0].enable_delay_from_src(DelayInp.NEXT_ALU_OUT_A, 2)
    B[1].enable_alu(AluOp.BYPASS, PD[1])                     # provider (c)
    B[1].alu_out_a_enable = 1
    B[1].pass_through_delay(0, 1, 2)
    B[1].enable_delay_from_src(DelayInp.PREV_ALU_OUT, 3)     # c3 <- p
    B[2].enable_alu(AluOp.MAX, PD[2], PD[0])                 # q = max(a,b)
    B[2].pass_through_delay(1, 3)
    B[3].enable_alu(AluOp.MIN, PREV, PD[1])                  # t = min(q,c)
    B[3].pass_through_delay(3)
    B[4].enable_alu(AluOp.MAX, PD[3], PREV)                  # mid = max(p,t)
    _finish_bypass(u, 5)
    u.enable_output(OutSel.ALU_OUT, OutPath.WR0_LO)
    return u


def build_h1() -> UopConfig:
    # lanes: 1 = SRC_0 (lo_k), 2 = SRC_0_HI (hi_k),
    #        3 = SRC_1 (lo_k1), 4 = SRC_1_HI (hi_k1)
    u = _steady(rd1=True)
    u.enable_input(InpSel.SRC_0, 1)
    u.enable_input(InpSel.SRC_0_HI, 2)
    u.enable_input(InpSel.SRC_1, 3)
    u.enable_input(InpSel.SRC_1_HI, 4)
    B = u.datapath_config
    # chains at blk0: c0<-lo_k, c1<-hi_k, c2<-lo_k1, c3<-hi_k1
    B[0].enable_alu(AluOp.MAX, NEXTA, PD[0])   # m1 = max(lo_{k-1}, lo_k)
    B[0].pass_through_delay(0, 1, 2, 3)
    # blk1: provider BYPASS(lo_k1) -> blk0's NEXT_A = lo_{k-1}
    B[1].enable_alu(AluOp.BYPASS, PD[2])
    B[1].alu_out_a_enable = 1
    B[1].pass_through_delay(1, 2, 3)
    B[1].enable_delay_from_src(DelayInp.NEXT_ALU_OUT_A, 4)   # c4 <- hi_{k-1}
    B[1].enable_delay_from_src(DelayInp.PREV_ALU_OUT, 5)     # c5 <- m1
    # blk2: provider BYPASS(hi_k1) -> blk1's c4 capture
    B[2].enable_alu(AluOp.BYPASS, PD[3])
    B[2].alu_out_a_enable = 1
    B[2].pass_through_delay(1, 2, 3, 4, 5)
    # blk3: A = max(m1, lo_k1)
    B[3].enable_alu(AluOp.MAX, PD[5], PD[2])
    B[3].pass_through_delay(1, 3, 4)
    # blk4: m2 = min(hi_{k-1}, hi_k)
    B[4].enable_alu(AluOp.MIN, PD[4], PD[1])
    B[4].pass_through_delay(3)
    B[4].enable_delay_from_src(DelayInp.PREV_ALU_OUT, 0)     # c0 <- A
    # blk5: C = min(m2, hi_k1)
    B[5].enable_alu(AluOp.MIN, PREV, PD[3])
    B[5].pass_through_delay(0)
    _finish_bypass(u, 6, carry=(0,))
    u.enable_output(OutSel.DELAY_0, OutPath.WR0_LO)   # A
    u.enable_output(OutSel.ALU_OUT, OutPath.WR0_HI)   # C
    return u


def build_h2a() -> UopConfig:
    # lanes: 1 = SRC_0 (z = m_k), 2 = SRC_1 (y = m_{k+1}); x = m_{k-1}
    # B = med3(x, y, z) = max(min(x,y), min(z, max(x,y)))
    u = _steady(rd1=True)
    u.enable_input(InpSel.SRC_0, 1)
    u.enable_input(InpSel.SRC_1, 2)
    B = u.datapath_config
    B[0].enable_alu(AluOp.MIN, NEXTA, PD[1])                 # p = min(x,y)
    B[0].pass_through_delay(0, 1)
    B[0].enable_delay_from_src(DelayInp.NEXT_ALU_OUT_A, 2)   # c2 <- x
    B[1].enable_alu(AluOp.BYPASS, PD[1])                     # provider (y)
    B[1].alu_out_a_enable = 1
    B[1].pass_through_delay(0, 1, 2)
    B[1].enable_delay_from_src(DelayInp.PREV_ALU_OUT, 3)     # c3 <- p
    B[2].enable_alu(AluOp.MAX, PD[2], PD[1])                 # q = max(x,y)
    B[2].pass_through_delay(0, 3)
    B[3].enable_alu(AluOp.MIN, PREV, PD[0])                  # t = min(q,z)
    B[3].pass_through_delay(3)
    B[4].enable_alu(AluOp.MAX, PD[3], PREV)                  # B = max(p,t)
    _finish_bypass(u, 5)
    u.enable_output(OutSel.ALU_OUT, OutPath.WR0_LO)
    return u


def build_h2b() -> UopConfig:
    # lanes: 1 = SRC_0 (A), 2 = SRC_0_HI (C), 3 = SRC_1 (Bm)
    # out = med3(A, Bm, C) = max(min(A,Bm), min(C, max(A,Bm)))
    u = _steady(rd1=True)
    u.enable_input(InpSel.SRC_0, 1)
    u.enable_input(InpSel.SRC_0_HI, 2)
    u.enable_input(InpSel.SRC_1, 3)
    B = u.datapath_config
    B[0].enable_alu(AluOp.MIN, PD[0], PD[2])                 # f1
    B[0].pass_through_delay(0, 1, 2)
    B[1].enable_alu(AluOp.MAX, PD[0], PD[2])                 # f2
    B[1].pass_through_delay(1)
    B[1].enable_delay_from_src(DelayInp.PREV_ALU_OUT, 3)     # c3 <- f1
    B[2].enable_alu(AluOp.MIN, PREV, PD[1])                  # f3
    B[2].pass_through_delay(3)
    B[3].enable_alu(AluOp.MAX, PD[3], PREV)                  # out
    _finish_bypass(u, 4)
    u.enable_output(OutSel.ALU_OUT, OutPath.WR0_LO)
    return u


# --- diagnostics ----------------------------------------------------------- #

def _build_h3(op3: AluOp) -> UopConfig:
    # out[e] = op3(in1[e-2], in1[e], in0[e]) -- with in1 = stream@(e+1) this
    # is op3 over the window {e-1, e, e+1}.  Same proven skeleton as H2A.
    u = _steady(rd1=True)
    u.enable_input(InpSel.SRC_0, 1)   # z = in0[e]
    u.enable_input(InpSel.SRC_1, 2)   # y = in1[e]
    B = u.datapath_config
    B[0].enable_alu(op3, NEXTA, PD[1])                       # m1 = op(x, y)
    B[0].pass_through_delay(0, 1)
    B[1].enable_alu(AluOp.BYPASS, PD[1])                     # provider (y)
    B[1].alu_out_a_enable = 1
    B[1].pass_through_delay(0)
    B[1].enable_delay_from_src(DelayInp.PREV_ALU_OUT, 3)     # c3 <- m1
    B[2].enable_alu(op3, PD[3], PD[0])                       # op(m1, z)
    _finish_bypass(u, 3)
    u.enable_output(OutSel.ALU_OUT, OutPath.WR0_LO)
    return u


def build_hmax3() -> UopConfig:
    return _build_h3(AluOp.MAX)


def build_hmin3() -> UopConfig:
    return _build_h3(AluOp.MIN)


def build_addraw() -> UopConfig:
    u = _steady(rd1=True)
    u.enable_input(InpSel.SRC_0, 1)
    u.enable_input(InpSel.SRC_1, 2)
    B = u.datapath_config
    B[0].enable_alu(AluOp.ADD, PD[0], PD[1])
    _finish_bypass(u, 1)
    u.enable_output(OutSel.ALU_OUT, OutPath.WR0_LO)
    return u


def build_tap2() -> UopConfig:
    # out[k] = in0[k] + in1[k-2]
    u = _steady(rd1=True)
    u.enable_input(InpSel.SRC_0, 1)
    u.enable_input(InpSel.SRC_1, 2)
    B = u.datapath_config
    B[0].enable_alu(AluOp.ADD, NEXTA, PD[0])
    B[0].pass_through_delay(1)
    B[1].enable_alu(AluOp.BYPASS, PD[1])
    B[1].alu_out_a_enable = 1
    _finish_bypass(u, 2)
    u.enable_output(OutSel.ALU_OUT, OutPath.WR0_LO)
    return u


def build_pairmm() -> UopConfig:
    # in0 = bf16 pairs (a, b); out = bf16 pairs (min(a,b), max(a,b))
    u = _steady(rd1=False)
    u.enable_input(InpSel.SRC_0, 1)
    u.enable_input(InpSel.SRC_0_HI, 2)
    B = u.datapath_config
    B[0].enable_alu(AluOp.MIN, PD[0], PD[1])
    B[0].pass_through_delay(0, 1)
    B[1].enable_alu(AluOp.MAX, PD[0], PD[1])
    B[1].enable_delay_from_src(DelayInp.PREV_ALU_OUT, 2)     # c2 <- min
    _finish_bypass(u, 2, carry=(2,))
    u.enable_output(OutSel.DELAY_2, OutPath.WR0_LO)   # min
    u.enable_output(OutSel.ALU_OUT, OutPath.WR0_HI)   # max
    return u


_PLACE2 = Spec(body=Src0 + Src1,
               reference=lambda in0, in1, s0, s1, imm2: in0 + in1)
_PLACE1 = Spec(body=Src0 + Src0,
               reference=lambda in0, in1, s0, s1, imm2: in0 + in0)

_BUILders = {
    "MED_V1_ANT": (build_v1, True, _PLACE2),
    "MED_V2_ANT": (build_v2, True, _PLACE2),
    "MED_H1_ANT": (build_h1, True, _PLACE2),
    "MED_H2A_ANT": (build_h2a, True, _PLACE2),
    "MED_H2B_ANT": (build_h2b, True, _PLACE2),
    "MED_ADDRAW_ANT": (build_addraw, True, _PLACE2),
    "MED_TAP2_ANT": (build_tap2, True, _PLACE2),
    "MED_PAIRMM_ANT": (build_pairmm, False, _PLACE1),
    "MED_HMAX3_ANT": (build_hmax3, True, _PLACE2),
    "MED_HMIN3_ANT": (build_hmin3, True, _PLACE2),
}

_REGISTERED: dict[str, RawDveOp] = {}


def _register_ops() -> dict[str, RawDveOp]:
    """Register the ops into dve_ops.OPS (idempotent); return name->op."""
    if _REGISTERED:
        return _REGISTERED
    for name, (build, rd1, spec) in _BUILders.items():
        existing = [o for o in dve_ops.OPS if o.name == name]
        if existing:
            _REGISTERED[name] = existing[0]
            continue
        op = RawDveOp(name=name, spec=spec, uops=(build(),), rd1_en=rd1)
        dve_ops.OPS.append(op)
        dve_ops._SUB_OPCODE_FOR_NAME[name] = (
            dve_ops._CUSTOM_DVE_ROW_BASE + len(dve_ops.OPS) - 1
        )
        dve_ops.CUSTOM_DVE_SPECS[name] = op.spec
        _REGISTERED[name] = op
    assert max(dve_ops._SUB_OPCODE_FOR_NAME.values()) < 0x20
    return _REGISTERED


# numpy references ---------------------------------------------------------- #

def np_bf16(x):
    import jax.numpy as jnp
    return np.asarray(jnp.asarray(x, dtype=jnp.bfloat16).astype(jnp.float32))


def ref_sort3(a, b, c):
    lo = np.minimum(np.minimum(a, b), c)
    hi = np.maximum(np.maximum(a, b), c)
    mid = np.maximum(np.minimum(a, b),
                     np.minimum(np.maximum(a, b), c))
    return lo, mid, hi




import os

import numpy as np

N_CORES = 8
B_FULL = 64
B_LOCAL = B_FULL // N_CORES
H = 224
W = 224
C = 3
WC = W * C                # 672
BANDS = 16
BAND_ROWS = H // BANDS    # 14
S = 7                     # output rows per chunk
SR = S + 1                # stream rows incl warmup dummy row 0
TROWS = 11                # xin tile rows (t=1..9 loaded)
PW = 684                  # padded interleaved row width
HN = 227                  # horizontal stream length per channel
AW = 228                  # per-channel plane width (even)

IMG_STRIDE = H * WC
BAND_STRIDE = BAND_ROWS * WC

LAST_RESULT = None


def _build_bass(repeat=1):
    import concourse.bacc as bacc
    import concourse.bass as bass
    import concourse.mybir as mybir
    import concourse.tile as tile

    OPS = _register_ops()

    f32 = mybir.dt.float32
    b16 = mybir.dt.bfloat16
    MIN = mybir.AluOpType.min
    MAX = mybir.AluOpType.max
    AP = bass.AP

    nc = bacc.Bacc("TRN2", target_bir_lowering=False, debug=False)

    x = nc.dram_tensor("x", [B_LOCAL, H, W, C], f32, kind="ExternalInput")
    y = nc.dram_tensor("y", [B_LOCAL, H, W, C], f32, kind="ExternalOutput")
    xt = x.ap().tensor
    yt = y.ap().tensor

    dummy = None
    if repeat != 1:
        dummy = nc.dram_tensor(f"dummyb{repeat}", [1, 128 + repeat], f32,
                               kind="ExternalInput")

    def dram_ap(t, offset, ap):
        return bass.AP(tensor=t, offset=offset, ap=ap)

    def sub(t_ap, delta, dims):
        return AP(tensor=t_ap.tensor, offset=t_ap.offset + delta,
                  ap=[t_ap.ap[0]] + dims)

    with tile.TileContext(nc) as tc:
        with (
            tc.tile_pool(name="pin", bufs=2) as pin,
            tc.tile_pool(name="pb", bufs=1) as pb,
            tc.tile_pool(name="pw", bufs=1) as pw,
            tc.tile_pool(name="pout", bufs=2) as pout,
        ):
            if dummy is not None:
                with tc.tile_pool(name="pd", bufs=1) as pd:
                    dt_ = pd.tile([1, 128 + repeat], f32, tag="dummy")
                    nc.sync.dma_start(out=dt_, in_=dummy.ap())

            def cd(op, out, in0, in1):
                nc.vector._custom_dve(OPS[op], out=out, in0=in0, in1=in1)

            for _rep in range(repeat):
                for chunk in range(2):
                    r0 = chunk * S
                    xin = pin.tile([128, TROWS, WC], f32, tag="xin")

                    # ---- load rows r0-1..r0+7 into tile rows 1..9 ----
                    if chunk == 0:
                        nc.sync.dma_start(
                            out=xin[8:128, 1:10, :],
                            in_=dram_ap(xt, BAND_STRIDE - WC,
                                        [[BAND_STRIDE, BANDS - 1],
                                         [IMG_STRIDE, B_LOCAL],
                                         [1, 9 * WC]]),
                        )
                        nc.sync.dma_start(
                            out=xin[0:8, 2:10, :],
                            in_=dram_ap(xt, 0, [[IMG_STRIDE, B_LOCAL],
                                               [1, 8 * WC]]),
                        )
                        nc.sync.dma_start(
                            out=xin[0:8, 1:2, :],
                            in_=dram_ap(xt, 0, [[IMG_STRIDE, B_LOCAL],
                                               [1, WC]]),
                        )
                    else:
                        nc.sync.dma_start(
                            out=xin[0:120, 1:10, :],
                            in_=dram_ap(xt, 6 * WC,
                                        [[BAND_STRIDE, BANDS - 1],
                                         [IMG_STRIDE, B_LOCAL],
                                         [1, 9 * WC]]),
                        )
                        nc.sync.dma_start(
                            out=xin[120:128, 1:9, :],
                            in_=dram_ap(xt, 15 * BAND_STRIDE + 6 * WC,
                                        [[IMG_STRIDE, B_LOCAL],
                                         [1, 8 * WC]]),
                        )
                        nc.sync.dma_start(
                            out=xin[120:128, 9:10, :],
                            in_=dram_ap(xt, 15 * BAND_STRIDE + 13 * WC,
                                        [[IMG_STRIDE, B_LOCAL], [1, WC]]),
                        )

                    # ---- cast to bf16 (ACT) ----
                    xb = pb.tile([128, 10, WC], b16, tag="xb")
                    nc.scalar.copy(out=xb[:, 1:10, :], in_=xin[:, 1:10, :])

                    xu = xb[:, 1:8, :]
                    xm = xb[:, 2:9, :]
                    xd = xb[:, 3:10, :]

                    # ---- vertical sort3: 6 stock bf16 ops at 2x ----
                    av = pw.tile([128, S, WC], b16, tag="t0")
                    bv = pw.tile([128, S, WC], b16, tag="t1")
                    tt = pw.tile([128, S, WC], b16, tag="t2")
                    lo = pw.tile([128, SR * PW], b16, tag="t3")
                    hi = pw.tile([128, SR * PW], b16, tag="t4")
                    mid = pw.tile([128, SR * PW], b16, tag="t5")
                    loa, hia, mida = lo[:, :], hi[:, :], mid[:, :]
                    vdim = [[PW, S], [1, WC]]
                    lod = sub(loa, PW + 6, vdim)
                    hid = sub(hia, PW + 6, vdim)
                    midd = sub(mida, PW + 6, vdim)
                    nc.vector.tensor_tensor(av, xu, xm, MIN)
                    nc.vector.tensor_tensor(bv, xu, xm, MAX)
                    nc.vector.tensor_tensor(lod, av, xd, MIN)
                    nc.vector.tensor_tensor(ttd := tt[:, :, :], bv, xd, MIN)
                    nc.vector.tensor_tensor(hid, bv, xd, MAX)
                    nc.vector.tensor_tensor(midd, av, ttd, MAX)

                    # ---- edge pads (ACT), data rows only ----
                    pdim = [[PW, S], [1, 3]]
                    for ta in (loa, hia, mida):
                        nc.scalar.copy(out=sub(ta, PW + 0, pdim),
                                       in_=sub(ta, PW + 6, pdim))
                        nc.scalar.copy(out=sub(ta, PW + 3, pdim),
                                       in_=sub(ta, PW + 6, pdim))
                        nc.scalar.copy(out=sub(ta, PW + 678, pdim),
                                       in_=sub(ta, PW + 675, pdim))

                    # ---- horizontal custom 1x ops, per channel ----
                    # streams start on dummy row 0 (warmup absorber)
                    A = pw.tile([128, C * SR * AW], b16, tag="t6")
                    Cc = pw.tile([128, C * SR * AW], b16, tag="t7")
                    Bb = pw.tile([128, C * SR * AW], b16, tag="t8")
                    Aa, Ca, Ba = A[:, :], Cc[:, :], Bb[:, :]
                    hin = [[PW, SR], [3, HN]]
                    hout = [[AW, SR], [1, HN]]
                    for c in range(C):
                        for name, sa, da in (
                            ("MED_HMAX3_ANT", loa, Aa),
                            ("MED_HMIN3_ANT", hia, Ca),
                            ("MED_H2A_ANT", mida, Ba),
                        ):
                            cd(name,
                               out=sub(da, c * SR * AW, hout),
                               in0=sub(sa, c, hin),
                               in1=sub(sa, c + 3, hin))

                    # ---- final med3(A,B,C): 4 stock bf16 2x ops ----
                    g = pw.tile([128, C * SR * AW], b16, tag="t3")
                    q = pw.tile([128, C * SR * AW], b16, tag="t4")
                    h = pw.tile([128, C * SR * AW], b16, tag="t5")
                    med = pw.tile([128, C * SR * AW], b16, tag="t6")
                    nc.vector.tensor_tensor(g, A, Bb, MIN)
                    nc.vector.tensor_tensor(q, A, Bb, MAX)
                    nc.vector.tensor_tensor(h, Cc, q, MIN)
                    nc.vector.tensor_tensor(med, g, h, MAX)

                    # ---- cast + reinterleave to f32 y (ACT), store ----
                    yo = pout.tile([128, S, WC], f32, tag="yo")
                    ya = yo[:, :, :]
                    meda = med[:, :]
                    for c in range(C):
                        nc.scalar.copy(
                            out=sub(ya, c, [[WC, S], [C, W]]),
                            in_=sub(meda, c * SR * AW + AW + 2,
                                    [[AW, S], [1, W]]),
                        )
                    nc.sync.dma_start(
                        out=dram_ap(yt, r0 * WC,
                                    [[BAND_STRIDE, BANDS],
                                     [IMG_STRIDE, B_LOCAL], [1, S * WC]]),
                        in_=yo,
                    )

    nc.compile()
    return nc


_NC_CACHE = None


def kernel(x: np.ndarray) -> np.ndarray:
    global LAST_RESULT, _NC_CACHE
    from concourse.bass_utils import run_bass_kernel_spmd

    assert x.shape == (B_FULL, H, W, C), x.shape
    x = np.ascontiguousarray(np.asarray(x, dtype=np.float32))

    if _NC_CACHE is None:
        _NC_CACHE = _build_bass()
    nc = _NC_CACHE

    in_maps = [
        {"x": x[i * B_LOCAL:(i + 1) * B_LOCAL]} for i in range(N_CORES)
    ]
    res = run_bass_kernel_spmd(
        nc, in_maps, core_ids=list(range(N_CORES)), trace=False,
    )
    LAST_RESULT = res
    out = np.concatenate([r["y"] for r in res.results], axis=0)
    return out
